# revision 36
# baseline (speedup 1.0000x reference)
"""Cross-attention kernel for Trainium2, 8-core SPMD.

Sharding: core = b*4 + g  (b: batch of 2, g: head-group of 4 heads = 256
q/k/v feature cols). Wq/Wk/Wv column-sharded, Wo row-sharded; the Wo
all-reduce is done host-side when unsharding (sum of partials).

Device layout notes (per core):
  - activations kept feature-major ("transposed"): xnT/cnT [e, tok]
  - kT [d_loc, Tc] and v [Tc, d_loc] resident in SBUF (bf16)
  - scores computed transposed S^T[c, q] = kT.T-slices @ qT; softmax
    without max-subtraction (scores ~ N(0,1), exp is fp32-safe);
    denominator comes free from a ones-column appended to V, so
    attention output arrives as outT[d+1, q] with the den in row 64.
  - LN gamma and the score scale are folded into the weights host-side;
    beta terms become per-feature biases (cq/ck/cv).

Engine balance (vs the all-DVE/all-Act baseline):
  - LN standardize (z) runs on the gpsimd/Pool engine (SBUF->SBUF only;
    Pool cannot touch PSUM).
  - PSUM->SBUF transpose copies alternate DVE / scalar(Act).
  - k/q bias+cast copies run on Act (activation Identity with bias AP).
  - softmax exp is split: Act computes exact exp on the first ACT_C
    query columns; DVE computes the remaining columns with the
    Schraudolph bit trick (scores are pre-scaled by 128*log2(e), so
    bf16 bits = int16(s_hw + 16255.5), a piecewise-linear 2^x). The
    constant relative bias of the PWL exp cancels in softmax; the
    residual ripple (~2%) only affects 1/4 of the queries, keeping
    total rel err ~1e-2 vs the 2e-2 gate.
"""

import numpy as np
import ml_dtypes

import concourse.bass as bass
import concourse.tile as tile
from concourse import bacc, mybir
from concourse.bass_utils import run_bass_kernel_spmd

EMB = 1024
TX = 1024
TC = 8192
DL = 256          # per-core q/k/v cols (4 heads x 64)
N_CORES = 8

F32 = mybir.dt.float32
BF16 = mybir.dt.bfloat16
I16 = mybir.dt.int16
AF = mybir.AluOpType
ACTF = mybir.ActivationFunctionType
PSUM = bass.MemorySpace.PSUM
BF16_NP = ml_dtypes.bfloat16
EPS = 1e-5

# softmax exp handling: scores arrive pre-scaled by 128*log2(e)
F_SCORE = float(16.0 * np.log2(np.e))       # folded into Wq (incl. 1/sqrt(64))
EXP_SCALE = float(np.log(2.0) / 128.0)      # Act: e^s = exp(s_hw * EXP_SCALE)
B_MAGIC = 16255.5                           # DVE: bf16 bits = int16(s_hw + B)
ACT_C = 512                                 # query cols done exactly on Act


def _ln_stats(nc, stat_p, xt, eps_sb):
    """LayerNorm stats for [128, 1024] f32: returns (mean, rstd) APs."""
    st = stat_p.tile([128, 2, 6], F32)
    nc.vector.bn_stats(out=st[:, 0, :], in_=xt[:, 0:512])
    nc.vector.bn_stats(out=st[:, 1, :], in_=xt[:, 512:1024])
    mv = stat_p.tile([128, 2], F32)
    nc.vector.bn_aggr(out=mv, in_=st)
    std = stat_p.tile([128, 1], F32)
    nc.scalar.activation(out=std, in_=mv[:, 1:2], func=ACTF.Sqrt, bias=eps_sb[:, 0:1])
    rstd = stat_p.tile([128, 1], F32)
    nc.vector.reciprocal(out=rstd, in_=std)
    return mv, rstd


def _ln_to_bf16(nc, stat_p, zpool, xt, eps_sb):
    """LayerNorm (standardize only) [128, 1024] f32 -> bf16."""
    mv, rstd = _ln_stats(nc, stat_p, xt, eps_sb)
    z = zpool.tile([128, EMB], BF16)
    nc.vector.tensor_scalar(
        out=z, in0=xt, scalar1=mv[:, 0:1], scalar2=rstd,
        op0=AF.subtract, op1=AF.mult,
    )
    return z


def _transpose_1024(nc, tc, tp_ps, dst3d, z, ident_sb, col0):
    """PE-transpose z [128, 1024] into dst3d[:, ec, col0:col0+128] for ec in 0..7.

    The PSUM->SBUF copies run on Act to keep DVE free for LN work."""
    for eg in range(2):
        tp = tp_ps.tile([128, 512], BF16)
        for j in range(4):
            ec = eg * 4 + j
            nc.tensor.transpose(
                tp[:, j * 128:(j + 1) * 128], z[:, ec * 128:(ec + 1) * 128], ident_sb
            )
        src = tp[:].rearrange("p (a b) -> p a b", b=128)
        dst = dst3d[:, eg * 4:(eg + 1) * 4, col0:col0 + 128]
        nc.scalar.copy(out=dst, in_=src)


def build_nc():
    from contextlib import ExitStack

    nc = bacc.Bacc("TRN2", target_bir_lowering=False, debug=False,
                   num_devices=N_CORES)

    x_d = nc.dram_tensor("x", [TX, EMB], F32, kind="ExternalInput")
    ctx_d = nc.dram_tensor("ctx", [TC, EMB], F32, kind="ExternalInput")
    wq_d = nc.dram_tensor("wq", [128, 8, DL], BF16, kind="ExternalInput")
    wk_d = nc.dram_tensor("wk", [128, 8, DL], BF16, kind="ExternalInput")
    wv_d = nc.dram_tensor("wv", [128, 8, DL], BF16, kind="ExternalInput")
    wo_d = nc.dram_tensor("wo", [128, 2, EMB], BF16, kind="ExternalInput")
    cq_d = nc.dram_tensor("cq", [128, 2], F32, kind="ExternalInput")
    ck_d = nc.dram_tensor("ck", [128, 2], F32, kind="ExternalInput")
    cv_d = nc.dram_tensor("cv", [128, DL], F32, kind="ExternalInput")
    id_d = nc.dram_tensor("ident", [128, 128], BF16, kind="ExternalInput")
    y_d = nc.dram_tensor("y", [TX, EMB], F32, kind="ExternalOutput")

    with tile.TileContext(nc) as tc, ExitStack() as top:
        consts = top.enter_context(tc.tile_pool(name="consts", bufs=1))
        wq_sb = consts.tile([128, 8, DL], BF16)
        nc.sync.dma_start(out=wq_sb, in_=wq_d[:])
        wk_sb = consts.tile([128, 8, DL], BF16)
        nc.sync.dma_start(out=wk_sb, in_=wk_d[:])
        wv_sb = consts.tile([128, 8, DL], BF16)
        nc.sync.dma_start(out=wv_sb, in_=wv_d[:])
        wo_sb = consts.tile([128, 2, EMB], BF16)
        nc.sync.dma_start(out=wo_sb, in_=wo_d[:])
        cq_sb = consts.tile([128, 2], F32)
        nc.sync.dma_start(out=cq_sb, in_=cq_d[:])
        ck_sb = consts.tile([128, 2], F32)
        nc.sync.dma_start(out=ck_sb, in_=ck_d[:])
        cv_sb = consts.tile([128, DL], F32)
        nc.sync.dma_start(out=cv_sb, in_=cv_d[:])
        ident_sb = consts.tile([128, 128], BF16)
        nc.sync.dma_start(out=ident_sb, in_=id_d[:])
        eps_sb = consts.tile([128, 1], F32)
        nc.vector.memset(eps_sb[:], EPS)
        bmagic_sb = consts.tile([128, 1], F32)
        nc.vector.memset(bmagic_sb[:], B_MAGIC)

        QT_sb = consts.tile([128, 2, TX], BF16)     # [d_in_ch, dch, q]

        # ---- long-lived K/V ----
        # kT is stored zero-padded per head: kT[dch][:, h2, :] has the head's
        # 64 dims on partitions h2*64..h2*64+63 and ZEROS on the other 64, so
        # the scores matmul contracts over K=128 (full-height PE tile, no
        # 64-row tile-mode switching between scores and attn@v matmuls).
        kv_pool = top.enter_context(tc.tile_pool(name="kv", bufs=1))
        kT = [kv_pool.tile([128, 2, TC], BF16, name=f"kT{i}") for i in range(2)]
        v_sb = kv_pool.tile([128, TC // 128, 4, 65], BF16)
        nc.vector.memset(v_sb[:, :, :, 64:65], 1.0)
        # zero the off-head partition ranges once (split DVE/Act)
        nc.vector.memset(kT[0][64:128, 0, :], 0.0)
        nc.vector.memset(kT[1][64:128, 0, :], 0.0)
        nc.scalar.memzero(kT[0][0:64, 1, :])
        nc.scalar.memzero(kT[1][0:64, 1, :])

        # ---- phases 1+2 fused: ctx -> kT,v with x -> QT interleaved (the
        # x tiles ride along with the first 8 ctx iterations, filling LN
        # latency bubbles; q-proj fires once xnT is complete) ----
        with ExitStack() as p2:
            cpool = p2.enter_context(tc.tile_pool(name="cp", bufs=6))
            zpool2 = p2.enter_context(tc.tile_pool(name="zp2", bufs=4))
            stat2 = p2.enter_context(tc.tile_pool(name="st2", bufs=8))
            cnT_p = p2.enter_context(tc.tile_pool(name="cnT", bufs=3))
            xpool = p2.enter_context(tc.tile_pool(name="xp", bufs=3))
            xnT_p = p2.enter_context(tc.tile_pool(name="xnT", bufs=1))
            tp_ps2 = p2.enter_context(tc.tile_pool(name="tps2", bufs=2, space=PSUM))
            kt_ps = p2.enter_context(tc.tile_pool(name="ktps", bufs=1, space=PSUM))
            v_ps = p2.enter_context(tc.tile_pool(name="vps", bufs=1, space=PSUM))
            qt_ps = p2.enter_context(tc.tile_pool(name="qtps", bufs=2, space=PSUM))
            xnT = xnT_p.tile([128, 8, TX], BF16)

            def emit_kvproj(ci, cnT):
                # accumulation chains interleaved pairwise so consecutive
                # matmuls hit different PSUM banks (avoids the same-bank
                # read-modify-write bubble, ~56ns per matmul)
                kps = [kt_ps.tile([128, 512], F32, name=f"kps{d}") for d in range(2)]
                for ec in range(8):
                    for dch in range(2):
                        nc.tensor.matmul(
                            kps[dch][:],
                            wk_sb[:, ec, dch * 128:(dch + 1) * 128],
                            cnT[:, ec, :],
                            start=(ec == 0), stop=(ec == 7),
                        )
                for dch in range(2):
                    for h2 in range(2):
                        pr = slice(h2 * 64, (h2 + 1) * 64)
                        nc.scalar.activation(
                            out=kT[dch][pr, h2, ci * 512:(ci + 1) * 512],
                            in_=kps[dch][pr, :], func=ACTF.Identity,
                            bias=ck_sb[pr, dch:dch + 1],
                        )
                for sp_ in range(2):
                    vps = [v_ps.tile([128, 256], F32, name=f"vps{j}") for j in range(2)]
                    for ec in range(8):
                        for j in range(2):
                            s = sp_ * 2 + j
                            nc.tensor.matmul(
                                vps[j][:],
                                cnT[:, ec, s * 128:(s + 1) * 128],
                                wv_sb[:, ec, :],
                                start=(ec == 0), stop=(ec == 7),
                            )
                    for j in range(2):
                        cc = ci * 4 + sp_ * 2 + j
                        nc.vector.tensor_add(
                            out=v_sb[:, cc, :, 0:64],
                            in0=vps[j][:].rearrange("p (h d) -> p h d", d=64),
                            in1=cv_sb[:].rearrange("p (h d) -> p h d", d=64),
                        )

            pending_kv = None   # (ci, cnT): k/v-proj lags the transpose stream
            for ci in range(16):
                cnT = cnT_p.tile([128, 8, 512], BF16)
                for s in range(4):
                    ct = cpool.tile([128, EMB], F32)
                    row = (ci * 4 + s) * 128
                    nc.sync.dma_start(out=ct, in_=ctx_d[row:row + 128, :])
                    z = _ln_to_bf16(nc, stat2, zpool2, ct, eps_sb)
                    _transpose_1024(nc, tc, tp_ps2, cnT, z, ident_sb, s * 128)
                if ci < 8:
                    xt = xpool.tile([128, EMB], F32)
                    nc.sync.dma_start(out=xt, in_=x_d[ci * 128:(ci + 1) * 128, :])
                    mv, rstd = _ln_stats(nc, stat2, xt, eps_sb)
                    nmr = stat2.tile([128, 1], F32)
                    nc.vector.tensor_scalar(out=nmr, in0=mv[:, 0:1], scalar1=rstd,
                                            scalar2=-1.0, op0=AF.mult, op1=AF.mult)
                    z = zpool2.tile([128, EMB], BF16, name="z")
                    nc.scalar.activation(out=z, in_=xt, func=ACTF.Identity,
                                         bias=nmr[:, 0:1], scale=rstd[:, 0:1])
                    _transpose_1024(nc, tc, tp_ps2, xnT, z, ident_sb, ci * 128)
                if ci == 8:
                    for dch in range(2):
                        for qh in range(2):
                            ps = qt_ps.tile([128, 512], F32)
                            for ec in range(8):
                                nc.tensor.matmul(
                                    ps[:],
                                    wq_sb[:, ec, dch * 128:(dch + 1) * 128],
                                    xnT[:, ec, qh * 512:(qh + 1) * 512],
                                    start=(ec == 0), stop=(ec == 7),
                                )
                            nc.scalar.activation(
                                out=QT_sb[:, dch, qh * 512:(qh + 1) * 512],
                                in_=ps[:], func=ACTF.Identity,
                                bias=cq_sb[:, dch:dch + 1],
                            )
                if pending_kv is not None:
                    emit_kvproj(*pending_kv)
                pending_kv = (ci, cnT)
            emit_kvproj(*pending_kv)

        # ---- phase 3: attention (two head-pair passes) ----
        # Software-pipelined by one (cc, h2) unit: attn@v for unit u-1 is
        # emitted after the scores of unit u, so the in-order PE queue never
        # stalls waiting for the exp of the unit it just produced.
        att_out = top.enter_context(tc.tile_pool(name="attout", bufs=1))
        outT_sb = att_out.tile([128, 2, TX], BF16)
        with ExitStack() as p3:
            sc_pa = p3.enter_context(tc.tile_pool(name="sca", bufs=2, space=PSUM))
            sc_pd = p3.enter_context(tc.tile_pool(name="scd", bufs=2, space=PSUM))
            pt_pa = p3.enter_context(tc.tile_pool(name="pta", bufs=4))
            pt_pd = p3.enter_context(tc.tile_pool(name="ptd", bufs=4))
            den_p = p3.enter_context(tc.tile_pool(name="den", bufs=1))

            def emit_scores(hp, cc, h2):
                # separate scores tiles and exp tiles per qh half, so the Act
                # (qh=0) and DVE (qh=1) exp paths share no tile at all
                spa = sc_pa.tile([128, 512], F32)
                spd = sc_pd.tile([128, 512], F32)
                for qh, sp in ((0, spa), (1, spd)):
                    nc.tensor.matmul(
                        sp[:],
                        kT[hp][:, h2, cc * 128:(cc + 1) * 128],
                        QT_sb[:, hp, qh * 512:(qh + 1) * 512],
                        start=True, stop=True,
                    )
                pa = pt_pa.tile([128, 512], BF16)
                nc.scalar.activation(
                    out=pa[:].bitcast(I16), in_=spa[:],
                    func=ACTF.Identity, bias=bmagic_sb[:, 0:1], scale=1.0,
                )
                pd = pt_pd.tile([128, 512], BF16)
                nc.vector.tensor_scalar_add(
                    out=pd[:].bitcast(I16), in0=spd[:], scalar1=B_MAGIC,
                )
                return pa, pd

            def emit_attnv(oT, hp, cc, h2, pts):
                h = hp * 2 + h2
                for qh in range(2):
                    nc.tensor.matmul(
                        oT[h2][0:65, qh * 512:(qh + 1) * 512],
                        v_sb[:, cc, h, :],
                        pts[qh][:],
                        start=(cc == 0), stop=(cc == 63),
                    )

            def emit_epilogue(oT, hp, h2, final):
                # Snapshot oT to SBUF first: the PSUM tiles' only reader is a
                # single fast copy, so the next hp pass's attn@v (WAR on oT)
                # can start ~1us later while the div chain runs overlapped.
                # The reciprocal of a single-partition row costs ~6.4ns/elem,
                # so both heads' dens are packed into one [2, TX] recip.
                # For the non-final pass the muls run on the otherwise-idle
                # gpsimd engine (slow but fully hidden behind the next pass);
                # keeping them off DVE keeps its in-order queue free for the
                # next pass's exp stream.
                o = den_p.tile([65, TX], F32, name=f"ocp{hp}_{h2}")
                nc.vector.tensor_copy(out=o, in_=oT[h2][0:65, :])
                r = den_p.tile([1, TX], F32, name=f"rec{hp}_{h2}")
                nc.vector.reciprocal(out=r, in_=o[64:65, :])
                rr = den_p.tile([64, TX], F32, name=f"rrep{hp}_{h2}")
                nc.gpsimd.partition_broadcast(rr[:], r[0:1, :])
                eng = nc.vector if final else nc.gpsimd
                eng.tensor_mul(
                    out=outT_sb[h2 * 64:(h2 + 1) * 64, hp, :],
                    in0=o[0:64, :], in1=rr,
                )

            # oT allocated once and reused across both hp passes: the second
            # pass's start=True matmuls reset PSUM, and reusing the tiles
            # avoids a pool-teardown all-engine barrier between passes.
            ot_ps = p3.enter_context(tc.tile_pool(name="ot", bufs=1, space=PSUM))
            oT = [ot_ps.tile([128, TX], F32, name=f"oT{i}") for i in range(2)]
            # h2-major order: head h2=0's oT finishes at the halfway point
            # of each hp pass, so its epilogue chain hides behind the h2=1
            # stream; only h2=1's chain remains at the pass boundary.
            for hp in range(2):
                pending = None   # (cc, h2, pt) awaiting attn@v
                for h2 in range(2):
                    for cc in range(64):
                        pt = emit_scores(hp, cc, h2)
                        if pending is not None:
                            emit_attnv(oT, hp, *pending)
                        pending = (cc, h2, pt)
                    if h2 == 0:
                        pt_last = pending
                        emit_attnv(oT, hp, *pt_last)
                        pending = None
                        emit_epilogue(oT, hp, h2=0, final=False)
                emit_attnv(oT, hp, *pending)
                emit_epilogue(oT, hp, h2=1, final=(hp == 1))

            # ---- phase 4 (inside the attention pool scope so no pool
            # teardown barrier precedes it): y = outT.T @ woP ----
            y_p = p3.enter_context(tc.tile_pool(name="ysb", bufs=3))
            for qt in range(8):
                ysb = y_p.tile([128, EMB], F32)
                for eh in range(2):
                    ps = (sc_pa.tile([128, 512], F32, name="spa") if eh == 0
                          else sc_pd.tile([128, 512], F32, name="spd"))
                    for dch in range(2):
                        nc.tensor.matmul(
                            ps[:],
                            outT_sb[:, dch, qt * 128:(qt + 1) * 128],
                            wo_sb[:, dch, eh * 512:(eh + 1) * 512],
                            start=(dch == 0), stop=(dch == 1),
                        )
                    nc.vector.tensor_copy(out=ysb[:, eh * 512:(eh + 1) * 512], in_=ps[:])
                nc.sync.dma_start(out=y_d[qt * 128:(qt + 1) * 128, :], in_=ysb)

    nc.compile()
    return nc


_NC_CACHE = []


def get_nc():
    if not _NC_CACHE:
        _NC_CACHE.append(build_nc())
    return _NC_CACHE[0]


def make_in_maps(inputs):
    x = np.asarray(inputs["x"], np.float32)
    context = np.asarray(inputs["context"], np.float32)
    Wq = np.asarray(inputs["Wq"], np.float32)
    Wk = np.asarray(inputs["Wk"], np.float32)
    Wv = np.asarray(inputs["Wv"], np.float32)
    Wo = np.asarray(inputs["Wo"], np.float32)
    g1 = np.asarray(inputs["g1"], np.float32)
    b1 = np.asarray(inputs["b1"], np.float32)
    g2 = np.asarray(inputs["g2"], np.float32)
    b2 = np.asarray(inputs["b2"], np.float32)
    ident = np.eye(128, dtype=BF16_NP)

    in_maps = []
    for core in range(N_CORES):
        b, g = core // 4, core % 4
        r = slice(g * DL, (g + 1) * DL)
        wqt = (F_SCORE * (g1[:, None] * Wq[r].T)).astype(BF16_NP)  # [1024, 256]
        wkt = (g2[:, None] * Wk[r].T).astype(BF16_NP)
        wvt = (g2[:, None] * Wv[r].T).astype(BF16_NP)
        wop = Wo[:, r].T.astype(BF16_NP)                           # [256, 1024]
        cq = (F_SCORE * (b1 @ Wq[r].T)).astype(np.float32)         # [256]
        ck = (b2 @ Wk[r].T).astype(np.float32)
        cv = (b2 @ Wv[r].T).astype(np.float32)
        in_maps.append({
            "x": np.ascontiguousarray(x[b]),
            "ctx": np.ascontiguousarray(context[b]),
            "wq": np.ascontiguousarray(wqt.reshape(8, 128, DL).transpose(1, 0, 2)),
            "wk": np.ascontiguousarray(wkt.reshape(8, 128, DL).transpose(1, 0, 2)),
            "wv": np.ascontiguousarray(wvt.reshape(8, 128, DL).transpose(1, 0, 2)),
            "wo": np.ascontiguousarray(wop.reshape(2, 128, EMB).transpose(1, 0, 2)),
            "cq": np.ascontiguousarray(cq.reshape(2, 128).T),
            "ck": np.ascontiguousarray(ck.reshape(2, 128).T),
            "cv": np.ascontiguousarray(np.tile(cv[None, :], (128, 1))),
            "ident": ident,
        })
    return in_maps


def unshard(results, inputs):
    bo = np.asarray(inputs["bo"], np.float32)
    ys = []
    for b in range(2):
        acc = results[b * 4 + 0]["y"].astype(np.float32).copy()
        for g in range(1, 4):
            acc += results[b * 4 + g]["y"]
        ys.append(acc + bo[None, :])
    return np.stack(ys, axis=0).astype(np.float32)


def kernel(**inputs):
    nc = get_nc()
    in_maps = make_in_maps(inputs)
    res = run_bass_kernel_spmd(nc, in_maps, core_ids=list(range(N_CORES)))
    return unshard(res.results, inputs)


# revision 37
# speedup vs baseline: 1.0217x; 1.0217x over previous
"""Cross-attention kernel for Trainium2, 8-core SPMD.

Sharding: core = b*4 + g  (b: batch of 2, g: head-group of 4 heads = 256
q/k/v feature cols). Wq/Wk/Wv column-sharded, Wo row-sharded; the Wo
all-reduce is done host-side when unsharding (sum of partials).

Device layout notes (per core):
  - activations kept feature-major ("transposed"): xnT/cnT [e, tok]
  - kT [d_loc, Tc] and v [Tc, d_loc] resident in SBUF (bf16)
  - scores computed transposed S^T[c, q] = kT.T-slices @ qT; softmax
    without max-subtraction (scores ~ N(0,1), exp is fp32-safe);
    denominator comes free from a ones-column appended to V, so
    attention output arrives as outT[d+1, q] with the den in row 64.
  - LN gamma and the score scale are folded into the weights host-side;
    beta terms become per-feature biases (cq/ck/cv).

Engine balance (vs the all-DVE/all-Act baseline):
  - LN standardize (z) runs on the gpsimd/Pool engine (SBUF->SBUF only;
    Pool cannot touch PSUM).
  - PSUM->SBUF transpose copies alternate DVE / scalar(Act).
  - k/q bias+cast copies run on Act (activation Identity with bias AP).
  - softmax exp is split: Act computes exact exp on the first ACT_C
    query columns; DVE computes the remaining columns with the
    Schraudolph bit trick (scores are pre-scaled by 128*log2(e), so
    bf16 bits = int16(s_hw + 16255.5), a piecewise-linear 2^x). The
    constant relative bias of the PWL exp cancels in softmax; the
    residual ripple (~2%) only affects 1/4 of the queries, keeping
    total rel err ~1e-2 vs the 2e-2 gate.
"""

import numpy as np
import ml_dtypes

import concourse.bass as bass
import concourse.tile as tile
from concourse import bacc, mybir
from concourse.bass_utils import run_bass_kernel_spmd

EMB = 1024
TX = 1024
TC = 8192
DL = 256          # per-core q/k/v cols (4 heads x 64)
N_CORES = 8

F32 = mybir.dt.float32
BF16 = mybir.dt.bfloat16
I16 = mybir.dt.int16
AF = mybir.AluOpType
ACTF = mybir.ActivationFunctionType
PSUM = bass.MemorySpace.PSUM
BF16_NP = ml_dtypes.bfloat16
EPS = 1e-5

# softmax exp handling: scores arrive pre-scaled by 128*log2(e)
F_SCORE = float(16.0 * np.log2(np.e))       # folded into Wq (incl. 1/sqrt(64))
EXP_SCALE = float(np.log(2.0) / 128.0)      # Act: e^s = exp(s_hw * EXP_SCALE)
B_MAGIC = 16255.5                           # DVE: bf16 bits = int16(s_hw + B)
ACT_C = 512                                 # query cols done exactly on Act


def _ln_stats(nc, stat_p, xt, eps_sb):
    """LayerNorm stats for [128, 1024] f32: returns (mean, rstd) APs."""
    st = stat_p.tile([128, 2, 6], F32)
    nc.vector.bn_stats(out=st[:, 0, :], in_=xt[:, 0:512])
    nc.vector.bn_stats(out=st[:, 1, :], in_=xt[:, 512:1024])
    mv = stat_p.tile([128, 2], F32)
    nc.vector.bn_aggr(out=mv, in_=st)
    std = stat_p.tile([128, 1], F32)
    nc.scalar.activation(out=std, in_=mv[:, 1:2], func=ACTF.Sqrt, bias=eps_sb[:, 0:1])
    rstd = stat_p.tile([128, 1], F32)
    nc.vector.reciprocal(out=rstd, in_=std)
    return mv, rstd


def _ln_to_bf16(nc, stat_p, zpool, xt, eps_sb):
    """LayerNorm (standardize only) [128, 1024] f32 -> bf16."""
    mv, rstd = _ln_stats(nc, stat_p, xt, eps_sb)
    z = zpool.tile([128, EMB], BF16)
    nc.vector.tensor_scalar(
        out=z, in0=xt, scalar1=mv[:, 0:1], scalar2=rstd,
        op0=AF.subtract, op1=AF.mult,
    )
    return z


def _transpose_1024(nc, tc, tp_ps, dst3d, z, ident_sb, col0):
    """PE-transpose z [128, 1024] into dst3d[:, ec, col0:col0+128] for ec in 0..7.

    The PSUM->SBUF copies run on Act to keep DVE free for LN work."""
    for eg in range(2):
        tp = tp_ps.tile([128, 512], BF16)
        for j in range(4):
            ec = eg * 4 + j
            nc.tensor.transpose(
                tp[:, j * 128:(j + 1) * 128], z[:, ec * 128:(ec + 1) * 128], ident_sb
            )
        src = tp[:].rearrange("p (a b) -> p a b", b=128)
        dst = dst3d[:, eg * 4:(eg + 1) * 4, col0:col0 + 128]
        nc.scalar.copy(out=dst, in_=src)


def build_nc():
    from contextlib import ExitStack

    nc = bacc.Bacc("TRN2", target_bir_lowering=False, debug=False,
                   num_devices=N_CORES)

    x_d = nc.dram_tensor("x", [TX, EMB], F32, kind="ExternalInput")
    ctx_d = nc.dram_tensor("ctx", [TC, EMB], F32, kind="ExternalInput")
    wq_d = nc.dram_tensor("wq", [128, 8, DL], BF16, kind="ExternalInput")
    wk_d = nc.dram_tensor("wk", [128, 8, DL], BF16, kind="ExternalInput")
    wv_d = nc.dram_tensor("wv", [128, 8, DL], BF16, kind="ExternalInput")
    wo_d = nc.dram_tensor("wo", [128, 2, EMB], BF16, kind="ExternalInput")
    cq_d = nc.dram_tensor("cq", [128, 2], F32, kind="ExternalInput")
    ck_d = nc.dram_tensor("ck", [128, 2], F32, kind="ExternalInput")
    cv_d = nc.dram_tensor("cv", [128, DL], F32, kind="ExternalInput")
    id_d = nc.dram_tensor("ident", [128, 128], BF16, kind="ExternalInput")
    y_d = nc.dram_tensor("y", [TX, EMB], F32, kind="ExternalOutput")

    with tile.TileContext(nc) as tc, ExitStack() as top:
        consts = top.enter_context(tc.tile_pool(name="consts", bufs=1))
        wq_sb = consts.tile([128, 8, DL], BF16)
        nc.sync.dma_start(out=wq_sb, in_=wq_d[:])
        wk_sb = consts.tile([128, 8, DL], BF16)
        nc.sync.dma_start(out=wk_sb, in_=wk_d[:])
        wv_sb = consts.tile([128, 8, DL], BF16)
        nc.sync.dma_start(out=wv_sb, in_=wv_d[:])
        wo_sb = consts.tile([128, 2, EMB], BF16)
        nc.sync.dma_start(out=wo_sb, in_=wo_d[:])
        cq_sb = consts.tile([128, 2], F32)
        nc.sync.dma_start(out=cq_sb, in_=cq_d[:])
        ck_sb = consts.tile([128, 2], F32)
        nc.sync.dma_start(out=ck_sb, in_=ck_d[:])
        cv_sb = consts.tile([128, DL], F32)
        nc.sync.dma_start(out=cv_sb, in_=cv_d[:])
        ident_sb = consts.tile([128, 128], BF16)
        nc.sync.dma_start(out=ident_sb, in_=id_d[:])
        eps_sb = consts.tile([128, 1], F32)
        nc.vector.memset(eps_sb[:], EPS)
        bmagic_sb = consts.tile([128, 1], F32)
        nc.vector.memset(bmagic_sb[:], B_MAGIC)

        QT_sb = consts.tile([128, 2, TX], BF16)     # [d_in_ch, dch, q]

        # ---- long-lived K/V ----
        # kT is stored zero-padded per head: kT[dch][:, h2, :] has the head's
        # 64 dims on partitions h2*64..h2*64+63 and ZEROS on the other 64, so
        # the scores matmul contracts over K=128 (full-height PE tile, no
        # 64-row tile-mode switching between scores and attn@v matmuls).
        kv_pool = top.enter_context(tc.tile_pool(name="kv", bufs=1))
        kT = [kv_pool.tile([128, 2, TC], BF16, name=f"kT{i}") for i in range(2)]
        v_sb = kv_pool.tile([128, TC // 128, 4, 65], BF16)
        nc.vector.memset(v_sb[:, :, :, 64:65], 1.0)
        # zero the off-head partition ranges once (split DVE/Act)
        nc.vector.memset(kT[0][64:128, 0, :], 0.0)
        nc.vector.memset(kT[1][64:128, 0, :], 0.0)
        nc.scalar.memzero(kT[0][0:64, 1, :])
        nc.scalar.memzero(kT[1][0:64, 1, :])

        # ---- phases 1+2 fused: ctx -> kT,v with x -> QT interleaved (the
        # x tiles ride along with the first 8 ctx iterations, filling LN
        # latency bubbles; q-proj fires once xnT is complete) ----
        with ExitStack() as p2:
            cpool = p2.enter_context(tc.tile_pool(name="cp", bufs=6))
            zpool2 = p2.enter_context(tc.tile_pool(name="zp2", bufs=4))
            stat2 = p2.enter_context(tc.tile_pool(name="st2", bufs=8))
            cnT_p = p2.enter_context(tc.tile_pool(name="cnT", bufs=3))
            xpool = p2.enter_context(tc.tile_pool(name="xp", bufs=3))
            xnT_p = p2.enter_context(tc.tile_pool(name="xnT", bufs=1))
            tp_ps2 = p2.enter_context(tc.tile_pool(name="tps2", bufs=2, space=PSUM))
            kt_ps = p2.enter_context(tc.tile_pool(name="ktps", bufs=1, space=PSUM))
            v_ps = p2.enter_context(tc.tile_pool(name="vps", bufs=1, space=PSUM))
            qt_ps = p2.enter_context(tc.tile_pool(name="qtps", bufs=2, space=PSUM))
            xnT = xnT_p.tile([128, 8, TX], BF16)

            def emit_kvproj(ci, cnT):
                # accumulation chains interleaved pairwise so consecutive
                # matmuls hit different PSUM banks (avoids the same-bank
                # read-modify-write bubble, ~56ns per matmul)
                kps = [kt_ps.tile([128, 512], F32, name=f"kps{d}") for d in range(2)]
                for ec in range(8):
                    for dch in range(2):
                        nc.tensor.matmul(
                            kps[dch][:],
                            wk_sb[:, ec, dch * 128:(dch + 1) * 128],
                            cnT[:, ec, :],
                            start=(ec == 0), stop=(ec == 7),
                        )
                for dch in range(2):
                    for h2 in range(2):
                        pr = slice(h2 * 64, (h2 + 1) * 64)
                        nc.scalar.activation(
                            out=kT[dch][pr, h2, ci * 512:(ci + 1) * 512],
                            in_=kps[dch][pr, :], func=ACTF.Identity,
                            bias=ck_sb[pr, dch:dch + 1],
                        )
                for sp_ in range(2):
                    vps = [v_ps.tile([128, 256], F32, name=f"vps{j}") for j in range(2)]
                    for ec in range(8):
                        for j in range(2):
                            s = sp_ * 2 + j
                            nc.tensor.matmul(
                                vps[j][:],
                                cnT[:, ec, s * 128:(s + 1) * 128],
                                wv_sb[:, ec, :],
                                start=(ec == 0), stop=(ec == 7),
                            )
                    for j in range(2):
                        cc = ci * 4 + sp_ * 2 + j
                        nc.vector.tensor_add(
                            out=v_sb[:, cc, :, 0:64],
                            in0=vps[j][:].rearrange("p (h d) -> p h d", d=64),
                            in1=cv_sb[:].rearrange("p (h d) -> p h d", d=64),
                        )

            pending_kv = None   # (ci, cnT): k/v-proj lags the transpose stream
            for ci in range(16):
                cnT = cnT_p.tile([128, 8, 512], BF16)
                for s in range(4):
                    ct = cpool.tile([128, EMB], F32)
                    row = (ci * 4 + s) * 128
                    nc.sync.dma_start(out=ct, in_=ctx_d[row:row + 128, :])
                    z = _ln_to_bf16(nc, stat2, zpool2, ct, eps_sb)
                    _transpose_1024(nc, tc, tp_ps2, cnT, z, ident_sb, s * 128)
                if ci < 8:
                    xt = xpool.tile([128, EMB], F32)
                    nc.sync.dma_start(out=xt, in_=x_d[ci * 128:(ci + 1) * 128, :])
                    mv, rstd = _ln_stats(nc, stat2, xt, eps_sb)
                    nmr = stat2.tile([128, 1], F32)
                    nc.vector.tensor_scalar(out=nmr, in0=mv[:, 0:1], scalar1=rstd,
                                            scalar2=-1.0, op0=AF.mult, op1=AF.mult)
                    z = zpool2.tile([128, EMB], BF16, name="z")
                    nc.scalar.activation(out=z, in_=xt, func=ACTF.Identity,
                                         bias=nmr[:, 0:1], scale=rstd[:, 0:1])
                    _transpose_1024(nc, tc, tp_ps2, xnT, z, ident_sb, ci * 128)
                if ci == 8:
                    for dch in range(2):
                        for qh in range(2):
                            ps = qt_ps.tile([128, 512], F32)
                            for ec in range(8):
                                nc.tensor.matmul(
                                    ps[:],
                                    wq_sb[:, ec, dch * 128:(dch + 1) * 128],
                                    xnT[:, ec, qh * 512:(qh + 1) * 512],
                                    start=(ec == 0), stop=(ec == 7),
                                )
                            nc.scalar.activation(
                                out=QT_sb[:, dch, qh * 512:(qh + 1) * 512],
                                in_=ps[:], func=ACTF.Identity,
                                bias=cq_sb[:, dch:dch + 1],
                            )
                if pending_kv is not None:
                    emit_kvproj(*pending_kv)
                pending_kv = (ci, cnT)
            emit_kvproj(*pending_kv)

        # ---- phase 3: attention (two head-pair passes) ----
        # Software-pipelined by one (cc, h2) unit: attn@v for unit u-1 is
        # emitted after the scores of unit u, so the in-order PE queue never
        # stalls waiting for the exp of the unit it just produced.
        att_out = top.enter_context(tc.tile_pool(name="attout", bufs=1))
        outT_sb = att_out.tile([128, 2, TX], BF16)
        with ExitStack() as p3:
            sc_pa = p3.enter_context(tc.tile_pool(name="sca", bufs=2, space=PSUM))
            sc_pd = p3.enter_context(tc.tile_pool(name="scd", bufs=2, space=PSUM))
            pt_pa = p3.enter_context(tc.tile_pool(name="pta", bufs=4))
            pt_pd = p3.enter_context(tc.tile_pool(name="ptd", bufs=4))
            den_p = p3.enter_context(tc.tile_pool(name="den", bufs=1))

            def emit_scores(hp, cc, h2):
                # separate scores tiles and exp tiles per qh half, so the Act
                # (qh=0) and DVE (qh=1) exp paths share no tile at all
                spa = sc_pa.tile([128, 512], F32)
                spd = sc_pd.tile([128, 512], F32)
                for qh, sp in ((0, spa), (1, spd)):
                    nc.tensor.matmul(
                        sp[:],
                        kT[hp][:, h2, cc * 128:(cc + 1) * 128],
                        QT_sb[:, hp, qh * 512:(qh + 1) * 512],
                        start=True, stop=True,
                    )
                pa = pt_pa.tile([128, 512], BF16)
                nc.scalar.activation(
                    out=pa[:].bitcast(I16), in_=spa[:],
                    func=ACTF.Identity, bias=bmagic_sb[:, 0:1], scale=1.0,
                )
                pd = pt_pd.tile([128, 512], BF16)
                nc.vector.tensor_scalar_add(
                    out=pd[:].bitcast(I16), in0=spd[:], scalar1=B_MAGIC,
                )
                return pa, pd

            def emit_attnv(oT, hp, cc, h2, pts):
                h = hp * 2 + h2
                for qh in range(2):
                    nc.tensor.matmul(
                        oT[h2][0:65, qh * 512:(qh + 1) * 512],
                        v_sb[:, cc, h, :],
                        pts[qh][:],
                        start=(cc == 0), stop=(cc == 63),
                    )

            def emit_epilogue(oT, hp, h2, final):
                # Snapshot oT to SBUF first: the PSUM tiles' only reader is a
                # single fast copy, so the next hp pass's attn@v (WAR on oT)
                # can start ~1us later while the div chain runs overlapped.
                # The reciprocal of a single-partition row costs ~6.4ns/elem,
                # so both heads' dens are packed into one [2, TX] recip.
                # For the non-final pass the muls run on the otherwise-idle
                # gpsimd engine (slow but fully hidden behind the next pass);
                # keeping them off DVE keeps its in-order queue free for the
                # next pass's exp stream.
                # DVE-free: snapshot + 1/den = exp(-ln(den)) run on Act (table
                # accuracy ~8e-5), broadcast+mul on gpsimd; only the final
                # boundary's mul uses DVE for a short tail.
                o = den_p.tile([65, TX], F32, name=f"ocp{hp}_{h2}")
                nc.scalar.copy(out=o, in_=oT[h2][0:65, :])
                ld = den_p.tile([1, TX], F32, name=f"ld{hp}_{h2}")
                nc.scalar.activation(out=ld, in_=o[64:65, :], func=ACTF.Ln)
                r = den_p.tile([1, TX], F32, name=f"rec{hp}_{h2}")
                nc.scalar.activation(out=r, in_=ld, func=ACTF.Exp, scale=-1.0)
                rr = den_p.tile([64, TX], F32, name=f"rrep{hp}_{h2}")
                nc.gpsimd.partition_broadcast(rr[:], r[0:1, :])
                eng = nc.vector if final else nc.gpsimd
                eng.tensor_mul(
                    out=outT_sb[h2 * 64:(h2 + 1) * 64, hp, :],
                    in0=o[0:64, :], in1=rr,
                )

            # oT allocated once and reused across both hp passes: the second
            # pass's start=True matmuls reset PSUM, and reusing the tiles
            # avoids a pool-teardown all-engine barrier between passes.
            ot_ps = p3.enter_context(tc.tile_pool(name="ot", bufs=1, space=PSUM))
            oT = [ot_ps.tile([128, TX], F32, name=f"oT{i}") for i in range(2)]
            # h2-major order: head h2=0's oT finishes at the halfway point
            # of each hp pass, so its epilogue chain hides behind the h2=1
            # stream; only h2=1's chain remains at the pass boundary.
            for hp in range(2):
                pending = None   # (cc, h2, pt) awaiting attn@v
                for h2 in range(2):
                    for cc in range(64):
                        pt = emit_scores(hp, cc, h2)
                        if pending is not None:
                            emit_attnv(oT, hp, *pending)
                        pending = (cc, h2, pt)
                    if h2 == 0:
                        pt_last = pending
                        emit_attnv(oT, hp, *pt_last)
                        pending = None
                        emit_epilogue(oT, hp, h2=0, final=False)
                emit_attnv(oT, hp, *pending)
                emit_epilogue(oT, hp, h2=1, final=(hp == 1))

            # ---- phase 4 (inside the attention pool scope so no pool
            # teardown barrier precedes it): y = outT.T @ woP ----
            y_p = p3.enter_context(tc.tile_pool(name="ysb", bufs=3))
            for qt in range(8):
                ysb = y_p.tile([128, EMB], F32)
                for eh in range(2):
                    ps = (sc_pa.tile([128, 512], F32, name="spa") if eh == 0
                          else sc_pd.tile([128, 512], F32, name="spd"))
                    for dch in range(2):
                        nc.tensor.matmul(
                            ps[:],
                            outT_sb[:, dch, qt * 128:(qt + 1) * 128],
                            wo_sb[:, dch, eh * 512:(eh + 1) * 512],
                            start=(dch == 0), stop=(dch == 1),
                        )
                    nc.vector.tensor_copy(out=ysb[:, eh * 512:(eh + 1) * 512], in_=ps[:])
                nc.sync.dma_start(out=y_d[qt * 128:(qt + 1) * 128, :], in_=ysb)

    nc.compile()
    return nc


_NC_CACHE = []


def get_nc():
    if not _NC_CACHE:
        _NC_CACHE.append(build_nc())
    return _NC_CACHE[0]


def make_in_maps(inputs):
    x = np.asarray(inputs["x"], np.float32)
    context = np.asarray(inputs["context"], np.float32)
    Wq = np.asarray(inputs["Wq"], np.float32)
    Wk = np.asarray(inputs["Wk"], np.float32)
    Wv = np.asarray(inputs["Wv"], np.float32)
    Wo = np.asarray(inputs["Wo"], np.float32)
    g1 = np.asarray(inputs["g1"], np.float32)
    b1 = np.asarray(inputs["b1"], np.float32)
    g2 = np.asarray(inputs["g2"], np.float32)
    b2 = np.asarray(inputs["b2"], np.float32)
    ident = np.eye(128, dtype=BF16_NP)

    in_maps = []
    for core in range(N_CORES):
        b, g = core // 4, core % 4
        r = slice(g * DL, (g + 1) * DL)
        wqt = (F_SCORE * (g1[:, None] * Wq[r].T)).astype(BF16_NP)  # [1024, 256]
        wkt = (g2[:, None] * Wk[r].T).astype(BF16_NP)
        wvt = (g2[:, None] * Wv[r].T).astype(BF16_NP)
        wop = Wo[:, r].T.astype(BF16_NP)                           # [256, 1024]
        cq = (F_SCORE * (b1 @ Wq[r].T)).astype(np.float32)         # [256]
        ck = (b2 @ Wk[r].T).astype(np.float32)
        cv = (b2 @ Wv[r].T).astype(np.float32)
        in_maps.append({
            "x": np.ascontiguousarray(x[b]),
            "ctx": np.ascontiguousarray(context[b]),
            "wq": np.ascontiguousarray(wqt.reshape(8, 128, DL).transpose(1, 0, 2)),
            "wk": np.ascontiguousarray(wkt.reshape(8, 128, DL).transpose(1, 0, 2)),
            "wv": np.ascontiguousarray(wvt.reshape(8, 128, DL).transpose(1, 0, 2)),
            "wo": np.ascontiguousarray(wop.reshape(2, 128, EMB).transpose(1, 0, 2)),
            "cq": np.ascontiguousarray(cq.reshape(2, 128).T),
            "ck": np.ascontiguousarray(ck.reshape(2, 128).T),
            "cv": np.ascontiguousarray(np.tile(cv[None, :], (128, 1))),
            "ident": ident,
        })
    return in_maps


def unshard(results, inputs):
    bo = np.asarray(inputs["bo"], np.float32)
    ys = []
    for b in range(2):
        acc = results[b * 4 + 0]["y"].astype(np.float32).copy()
        for g in range(1, 4):
            acc += results[b * 4 + g]["y"]
        ys.append(acc + bo[None, :])
    return np.stack(ys, axis=0).astype(np.float32)


def kernel(**inputs):
    nc = get_nc()
    in_maps = make_in_maps(inputs)
    res = run_bass_kernel_spmd(nc, in_maps, core_ids=list(range(N_CORES)))
    return unshard(res.results, inputs)
